# revision 1
# baseline (speedup 1.0000x reference)
"""Bass/Trainium2 kernel for nn_BinaryLSTMCell (B=65536, D=U=256).

Strategy (data-parallel over 8 cores, 8192 batch rows each):
  - Host: binarize kernels, permute recurrent kernel columns so the
    r-projection accumulates gate-aligned with the x-projection
    (reference pairs f<-x_i+r_f, i<-x_f+r_i), pack inputs/h transposed
    so every device DMA is a flat [128, 2048] contiguous copy.
  - Device, per 128-row tile: x-matmuls -> PSUM, hard-tanh clamp of the
    PSUM in place (DVE), r-matmuls accumulate on top (PE has_written
    bits stay set from the x-matmuls so accumulation lands on the
    clamped values), ACT evicts pre-gates to SBUF with cast, GPSIMD
    clamps the gates, DVE runs the c/h elementwise chain slab-wise.
"""

import os
import sys

for _p in ("/opt/trn_rl_repo", "/root/.axon_site/_ro/trn_rl_repo"):
    if os.path.isdir(_p) and _p not in sys.path:
        sys.path.append(_p)

import numpy as np
from contextlib import ExitStack

import concourse.bass as bass
import concourse.bacc as bacc
import concourse.mybir as mybir
from concourse.tile import TileContext
from concourse.bass_utils import run_bass_kernel_spmd

F32 = mybir.dt.float32
F32R = mybir.dt.float32r
BF16 = mybir.dt.bfloat16
ALU = mybir.AluOpType

N_CORES = 8
B = 65536
D = 256
U = 256
B_CORE = B // N_CORES          # 8192
SUPER = 1024                   # batch rows per super-tile
T_PER_S = SUPER // 128         # 8 tiles of 128 rows per super-tile
N_SUPER = B_CORE // SUPER      # 8 super-tiles per core

# gate/intermediate dtype: bf16 keeps DVE in 2x/4x perf modes.
GATE_DT = BF16


def _clamp(eng, out_ap, in_ap):
    """out = min(max(in, -1), 1) as one fused tensor_scalar."""
    eng.tensor_scalar(out_ap, in_ap, -1.0, 1.0, ALU.max, ALU.min)


def build_program(n_super=N_SUPER, gate_dt=GATE_DT):
    ABL = os.environ.get("KABL", "")
    """Build the per-core SPMD Bass program.

    DRAM layouts (all per-core):
      x, h   : [n_super, 128, 2048]  packed transposed activations;
               free index = k*1024 + t*128 + q holds element
               [d = k*128 + p, batch row = s*1024 + 8*q + t]
      c      : [n_super, 128, 2048]  natural rows; partition q, free
               t*256 + u holds c[s*1024 + 8*q + t, u]
      wx, wr : [128, 2048] binarized weights; free = k*1024 + col
      hn, cn : like c (outputs)
    """
    nc = bacc.Bacc("TRN2", target_bir_lowering=False, debug=False)

    # const AP for bias=2.0 (used by the ACT Relu-chain hard-tanh)
    _c2 = nc.alloc_sbuf_tensor("const-float32-2x0", [128, 1], F32)
    nc.gpsimd.memset(_c2.ap(), 2.0)
    nc.const_aps.aps[(F32, 2.0)] = _c2.ap()
    nc.all_engine_barrier()

    x_d = nc.dram_tensor("x", [n_super, 128, 2048], F32R, kind="ExternalInput")
    h_d = nc.dram_tensor("h", [n_super, 128, 2048], F32R, kind="ExternalInput")
    c_d = nc.dram_tensor("c", [n_super, 128, 2048], BF16, kind="ExternalInput")
    wx_d = nc.dram_tensor("wx", [128, 2048], F32R, kind="ExternalInput")
    wr_d = nc.dram_tensor("wr", [128, 2048], F32R, kind="ExternalInput")
    hn_d = nc.dram_tensor("hn", [n_super, 128, 2048], BF16, kind="ExternalOutput")
    cn_d = nc.dram_tensor("cn", [n_super, 128, 2048], BF16, kind="ExternalOutput")

    with TileContext(nc) as tc, ExitStack() as ctx:
        wpool = ctx.enter_context(tc.tile_pool(name="w", bufs=1))
        iopool = ctx.enter_context(tc.tile_pool(name="io", bufs=2))
        inpool = ctx.enter_context(tc.tile_pool(name="inp", bufs=int(os.environ.get("KINBUFS", "3"))))
        mpool = ctx.enter_context(tc.tile_pool(name="mid", bufs=int(os.environ.get("KMIDBUFS", "2"))))
        _G = int(os.environ.get("KGROUP", "1"))
        pspool = ctx.enter_context(tc.tile_pool(name="ps", bufs=4 // _G, space="PSUM"))

        wx = wpool.tile([128, 2048], F32R, tag="wx")
        nc.sync.dma_start(wx[:], wx_d.ap()[:, :])
        wr = wpool.tile([128, 2048], F32R, tag="wr")
        nc.sync.dma_start(wr[:], wr_d.ap()[:, :])

        def stage_in(s):
            xt = inpool.tile([128, 2048], F32R, tag="xt", name=f"xt_{s}")
            nc.sync.dma_start(xt[:], x_d.ap()[s])
            ht = inpool.tile([128, 2048], F32R, tag="ht", name=f"ht_{s}")
            nc.sync.dma_start(ht[:], h_d.ap()[s])
            # h and c arrive pre-clamped from the host (hard_tanh is a
            # pure input transform); c casts to bf16 during the DMA (SWDGE)
            cc = mpool.tile([128, 2048], gate_dt, tag="cc", name=f"cc_{s}")
            nc.sync.dma_start(cc[:], c_d.ap()[s])
            return xt, ht, cc

        staged = {}
        pending_tail = {}
        for s in range(n_super):
            if s not in staged:
                staged[s] = stage_in(s)
            xt, ht, cc = staged.pop(s)

            gates = mpool.tile([128, T_PER_S, 1024], gate_dt, tag="gates")

            # float32r runs the PE at full rate (1 cyc/row vs 4 for fp32
            # when the moving dim >= 256); bits are fp32, PE rounds
            # internally. Operands are bitcast views; PSUM stays fp32.
            def emit_xproj(t, ps):
                for n in range(2):
                    o = ps[:, n * 512:(n + 1) * 512]
                    nc.tensor.matmul(
                        o,
                        (xt[:, t * 128:(t + 1) * 128]),
                        (wx[:, n * 512:(n + 1) * 512]),
                        start=True, stop=False,
                    )
                    nc.tensor.matmul(
                        o,
                        (xt[:, 1024 + t * 128:1024 + (t + 1) * 128]),
                        (wx[:, 1024 + n * 512:1024 + (n + 1) * 512]),
                        start=False, stop=True,
                    )

            def emit_rproj(t, ps):
                for n in range(2):
                    o = ps[:, n * 512:(n + 1) * 512]
                    nc.tensor.matmul(
                        o,
                        (ht[:, t * 128:(t + 1) * 128]),
                        (wr[:, n * 512:(n + 1) * 512]),
                        start=False, stop=False, skip_group_check=True,
                    )
                    nc.tensor.matmul(
                        o,
                        (ht[:, 1024 + t * 128:1024 + (t + 1) * 128]),
                        (wr[:, 1024 + n * 512:1024 + (n + 1) * 512]),
                        start=False, stop=(n == 1), skip_group_check=True,
                    )

            # Software-pipelined across tile-GROUPS (G tiles share one
            # PSUM tile) so PE streams longer matmul bursts between seams;
            # group g+1's x-MMs run while group g's PSUM clamp happens.
            G = int(os.environ.get("KGROUP", "1"))
            n_groups = T_PER_S // G
            pss = {}
            for g in range(n_groups + 1):
                if g < n_groups:
                    pss[g] = pspool.tile([128, 1024 * G], F32, tag="ps",
                                         name=f"ps_{s}_{g}")
                    for i in range(G):
                        emit_xproj(g * G + i, pss[g][:, i * 1024:(i + 1) * 1024])
                    if (g % T_PER_S) < int(os.environ.get("KXB", "0")):
                        # split clamp: DVE takes bank A (1-op), ACT takes
                        # bank B via the exact 3-op Relu chain, concurrently
                        _clamp(nc.vector, pss[g][:, 0:512], pss[g][:, 0:512])
                        p = pss[g][:, 512:1024]
                        AF = mybir.ActivationFunctionType
                        nc.scalar.activation(p, p, AF.Relu, bias=1.0, scale=1.0)
                        nc.scalar.activation(p, p, AF.Relu, bias=2.0, scale=-1.0)
                        nc.scalar.activation(p, p, AF.Copy, bias=1.0, scale=-1.0)
                    else:
                        _clamp(nc.vector, pss[g][:], pss[g][:])
                if g >= 1:
                    gp = g - 1
                    for i in range(G):
                        emit_rproj(gp * G + i, pss[gp][:, i * 1024:(i + 1) * 1024])
                    # evict pre-gates (cast) then clamp -> f,i,g,o
                    gslice = gates[:, gp * G:(gp + 1) * G, :]
                    nc.scalar.copy(
                        gslice,
                        pss[gp][:].rearrange("p (g u) -> p g u", u=1024))
                    if os.environ.get("KGCLAMP", "pool") == "dve":
                        _clamp(nc.vector, gslice, gslice)
                    else:
                        _clamp(nc.gpsimd, gslice, gslice)
                    del pss[gp]
                # previous super-tile's tail goes here so DVE serves this
                # super's first PSUM clamps (and PE) before the tail chain
                if g * G == int(os.environ.get("KDEFT", "6")) and (s - 1) in pending_tail:
                    pending_tail.pop(s - 1)()

            # prefetch + pre-clamp next super-tile before the tail chain
            # so PE's next r-matmuls never wait on the DVE h-clamp
            if s + 1 < n_super and os.environ.get("KPREF", "1") == "1":
                staged[s + 1] = stage_in(s + 1)

            # slab-wise elementwise; 3-D APs keep the free dim contiguous
            def g3(lo, hi, gates=gates):
                return gates[:, :, lo:hi]

            def s3(tile):
                return tile[:].rearrange("p (t u) -> p t u", u=256)

            t1 = mpool.tile([128, 2048], gate_dt, tag="t1", name=f"t1_{s}")
            t2 = mpool.tile([128, 2048], gate_dt, tag="t2", name=f"t2_{s}")
            z = mpool.tile([128, 2048], gate_dt, tag="z", name=f"z_{s}")
            out_dt = gate_dt if os.environ.get("KOUT", "bf16") == "bf16" else F32
            cnew = iopool.tile([128, 2048], out_dt, tag="cn", name=f"cn_{s}")
            hnew = iopool.tile([128, 2048], out_dt, tag="hn", name=f"hn_{s}")

            def emit_tail(s=s, gates=gates, cc=cc, t1=t1, t2=t2, z=z,
                          cnew=cnew, hnew=hnew, g3=g3):
                tteng = nc.gpsimd if os.environ.get("KTT", "dve") == "pool" else nc.vector
                zeng = nc.vector if os.environ.get("KZ", "dve") == "dve" else nc.gpsimd
                halves = int(os.environ.get("KHALVES", "1"))
                tp = T_PER_S // halves
                for hh in range(halves):
                    ts_ = slice(hh * tp, (hh + 1) * tp)
                    cs = slice(hh * tp * 256, (hh + 1) * tp * 256)
                    tteng.tensor_tensor(s3(t1)[:, ts_], g3(0, 256)[:, ts_], s3(cc)[:, ts_], ALU.mult)
                    t2eng = nc.gpsimd if os.environ.get("KT2", "dve") == "pool" else tteng
                    t2eng.tensor_tensor(s3(t2)[:, ts_], g3(256, 512)[:, ts_], g3(512, 768)[:, ts_], ALU.mult)
                    nc.vector.tensor_tensor(s3(cnew)[:, ts_], s3(t1)[:, ts_], s3(t2)[:, ts_], ALU.add)
                    _clamp(zeng, z[:, cs], cnew[:, cs])
                    # o, z in [-1,1] so the outer hard_tanh is the identity
                    nc.vector.tensor_tensor(s3(hnew)[:, ts_], g3(768, 1024)[:, ts_], s3(z)[:, ts_], ALU.mult)
                nc.sync.dma_start(hn_d.ap()[s], hnew[:])
                nc.sync.dma_start(cn_d.ap()[s], cnew[:])

            if os.environ.get("KDEFER", "1") == "1" and s + 1 < n_super:
                pending_tail[s] = emit_tail
            else:
                emit_tail()

        for f in list(pending_tail.values()):
            f()

    nc.compile()
    return nc


def _pack_activation(a_core):
    """[rows, 256] -> [n_super, 128, 2048] transposed+permuted layout."""
    n_super = a_core.shape[0] // SUPER
    # [s, q, t, k, p] from rows s*1024 + 8q + t, cols k*128 + p
    v = a_core.reshape(n_super, 128, 8, 2, 128)
    return np.ascontiguousarray(v.transpose(0, 4, 3, 2, 1)).reshape(
        n_super, 128, 2048)


def _pack_weight(w):
    """[256, 1024] -> [128, 2048] with free = k*1024 + col."""
    return np.ascontiguousarray(
        w.reshape(2, 128, 1024).transpose(1, 0, 2)).reshape(128, 2048)


_PROGRAM_CACHE = {}


def _get_program():
    key = (N_SUPER, GATE_DT)
    if key not in _PROGRAM_CACHE:
        _PROGRAM_CACHE[key] = build_program()
    return _PROGRAM_CACHE[key]


def _run(inputs, h, c, kernel_w, recurrent_kernel, trace=False):
    X = np.ascontiguousarray(np.asarray(inputs, dtype=np.float32))
    H = np.ascontiguousarray(np.asarray(h, dtype=np.float32))
    C = np.ascontiguousarray(np.asarray(c, dtype=np.float32))
    Wk = np.asarray(kernel_w, dtype=np.float32)
    Rk = np.asarray(recurrent_kernel, dtype=np.float32)

    Wb = np.where(Wk >= 0, np.float32(1.0), np.float32(-1.0))
    Rb = np.where(Rk >= 0, np.float32(1.0), np.float32(-1.0))
    # reorder r columns to [r_f, r_i, r_c, r_o] so PSUM accumulation is
    # gate-aligned (f pairs x_i with W_f, i pairs x_f with W_i)
    Rb = np.concatenate(
        [Rb[:, U:2 * U], Rb[:, 0:U], Rb[:, 2 * U:3 * U], Rb[:, 3 * U:]], axis=1)

    wx_np = _pack_weight(Wb)
    wr_np = _pack_weight(Rb)

    in_maps = []
    for m in range(N_CORES):
        lo, hi = m * B_CORE, (m + 1) * B_CORE
        in_maps.append({
            "x": _pack_activation(X[lo:hi]),
            "h": _pack_activation(np.clip(H[lo:hi], -1.0, 1.0)),
            "c": np.ascontiguousarray(np.clip(C[lo:hi], -1.0, 1.0).astype(
                mybir.dt.np(BF16))).reshape(N_SUPER, 128, 2048),
            "wx": wx_np,
            "wr": wr_np,
        })

    nc = _get_program()
    res = run_bass_kernel_spmd(nc, in_maps, core_ids=list(range(N_CORES)),
                               trace=trace)

    h_new = np.empty((B, U), dtype=np.float32)
    c_new = np.empty((B, U), dtype=np.float32)
    for m in range(N_CORES):
        lo, hi = m * B_CORE, (m + 1) * B_CORE
        h_new[lo:hi] = np.asarray(res.results[m]["hn"], dtype=np.float32).reshape(B_CORE, U)
        c_new[lo:hi] = np.asarray(res.results[m]["cn"], dtype=np.float32).reshape(B_CORE, U)
    return (h_new, h_new, c_new), res


def kernel(inputs, h, c, kernel, recurrent_kernel):
    outs, _ = _run(inputs, h, c, kernel, recurrent_kernel, trace=False)
    return outs

